# revision 19
# baseline (speedup 1.0000x reference)
"""Multi-head causal self-attention (B=2, N=2048, D=2048, H=16) on 8 NeuronCores.

Sharding: core c handles batch b = c//4 and heads 4*(c%4) .. 4*(c%4)+3
(data parallel over batch, tensor parallel over heads).  Each core:
  - projects V first (natural [seq, head_dim] layout), then Q^T / K^T in
    [head_dim, seq] layout, contracting over d_in chunks that sit on
    partitions (x is pre-transposed on the host),
  - runs causal attention per head entirely in transposed space
    (S^T = K_tile Q^T, exp on ScalarE writes P^T straight into SBUF,
    denominators via a ones-row matmul, 1/sum fused into the ctx copy),
  - computes the partial output projection ctx_slice @ W_out[rows_slice]
    into a [2048, 2048] fp32 partial.
The host sums the 4 partials per batch and adds the output bias.

v2 schedule: input DMAs are issued in consumption order in ~0.26 MB
chunks (16 DMA engines run in parallel; one huge DMA would serialize on
a single engine); the first two heads' g=3 score blocks + exp are woven
into the tail of the QK projection so the softmax pipeline is already
primed when projection ends; the causal mask rides the score PSUM
accumulation as an identity*maskT matmul (no cross-engine hop before
exp); exp is batched in [128,1024] pairs to amortize ACT access
latency; PSUM->SBUF out-tile copies run on DVE (ACT does only exp); and
out-projection tiles are interleaved as filler work between attention
steps so the PE never idles (idle costs ~3us of half-clock HAM ramp).

Matmul inputs are bf16 (fp32 accumulation in PSUM); measured end-to-end
relative error vs the fp32 reference is ~5.5e-3.
"""

import math

import numpy as np
import ml_dtypes

import concourse.bass as bass
import concourse.mybir as mybir
import concourse.tile as tile
from concourse import bacc
from concourse.bass_utils import run_bass_kernel_spmd

BF16 = mybir.dt.bfloat16
F32 = mybir.dt.float32
ALU = mybir.AluOpType
ACT_EXP = mybir.ActivationFunctionType.Exp

P = 128              # partitions
D_IN = 2048          # model dim
N_SEQ = 2048         # sequence length
HD = 128             # head dim
HPC = 4              # heads per core
DC = HPC * HD        # 512: d_out slice per core
N_CORES = 8
SCALE = 1.0 / math.sqrt(HD)
NEG_BIG = -1e10

NT = N_SEQ // P      # 16 seq tiles of 128
NI = D_IN // P       # 16 contraction chunks of 128
NG = NT // 4         # 4 groups of 4 q-tiles (q-512 groups)
NJ = D_IN // 512     # 4 output column chunks


def _build_body(tc, xt_d, wq_d, wk_d, wv_d, wo_d, out_d):
    nc = tc.nc
    from contextlib import ExitStack
    ctx = ExitStack()
    with ctx:
        # ---------------- constants ----------------
        const = ctx.enter_context(tc.tile_pool(name="const", bufs=1))
        # maskT[p=k, f=q] = -1e10 where q < k, else 0 (bf16, fed to the PE)
        maskT = const.tile([P, P], BF16)
        nc.gpsimd.memset(maskT, 0.0)
        nc.gpsimd.affine_select(
            out=maskT, in_=maskT, compare_op=ALU.is_ge, fill=NEG_BIG,
            base=0, pattern=[[1, P]], channel_multiplier=-1,
        )
        # identity (stationary for the mask-accumulate matmul)
        i128 = const.tile([P, P], BF16)
        nc.gpsimd.memset(i128, 1.0)
        nc.gpsimd.affine_select(
            out=i128, in_=i128, compare_op=ALU.is_equal, fill=0.0,
            base=0, pattern=[[1, P]], channel_multiplier=-1,
        )
        ones_sb = const.tile([P, 1], BF16)
        nc.vector.memset(ones_sb, 1.0)
        warmsrc = const.tile([P, 512], BF16)
        nc.vector.memset(warmsrc, 0.0)
        # force the exp activation table load at t=0, off the critical path
        tscr = const.tile([P, 1], F32)
        nc.vector.memset(tscr, 0.0)
        nc.scalar.activation(out=tscr, in_=tscr, func=ACT_EXP, bias=0.0, scale=1.0)

        # ---------------- persistent activations ----------------
        persist = ctx.enter_context(tc.tile_pool(name="persist", bufs=1))
        qt_sb = persist.tile([P, HPC, N_SEQ], BF16)    # Q^T  [d, h, n]
        kt_sb = persist.tile([P, HPC, N_SEQ], BF16)    # K^T  [d, h, n]
        v_sb = persist.tile([P, NT, DC], BF16)         # V natural [n(128), nt, d]
        ctxT_sb = persist.tile([P, HPC, N_SEQ], BF16)  # ctx^T [d, h, n]
        # dedicated P^T tile for the score step woven into projection
        wv_ptg0 = persist.tile([P, NT, 512], BF16)

        xt_r = xt_d.rearrange("(io p) nn -> p io nn", p=P)
        wq_r = wq_d.rearrange("(io p) c -> p io c", p=P)
        wk_r = wk_d.rearrange("(io p) c -> p io c", p=P)
        wv_r = wv_d.rearrange("(io p) c -> p io c", p=P)
        wo_r = wo_d.rearrange("(h p) j -> p h j", p=P)

        # step list: groups in descending size so the pipeline has big work
        # first; (g, h) pairs in emission order
        steps = [(3 - i // HPC, i % HPC) for i in range(NG * HPC)]
        LAG = 3

        # ---------------- score/exp emitter (shared proj + main) ----------
        def emit_scores_units(g, h, ptg, s_pool):
            """Yield after each PE unit: full-kt pairs then diag pairs.
            Scores for group g head h: S^T = K_kt.T @ Q^T, mask accumulated
            on the PE for diagonal blocks, batched exp into ptg (bf16)."""
            nkt = 4 * (g + 1)
            q0 = 512 * g
            nfull = 4 * g
            # full-width kt tiles, two per [128,1024] psum tensor with one
            # batched exp (amortizes ACT access latency)
            for kp in range(0, nfull, 2):
                sps = s_pool.tile([P, 1024], F32, tag="s", name="sps")
                for j, kt in enumerate((kp, kp + 1)):
                    nc.tensor.matmul(
                        sps[:, j * 512:(j + 1) * 512],
                        lhsT=kt_sb[:, h, kt * P:(kt + 1) * P],
                        rhs=qt_sb[:, h, q0:q0 + 512],
                        start=True, stop=True,
                    )
                nc.scalar.activation(
                    out=ptg[:, kp:kp + 2, :], in_=sps,
                    func=ACT_EXP, bias=0.0, scale=SCALE,
                )
                yield
            # diagonal kt tiles (off = 128*(kt-4g)), two per psum tensor,
            # causal mask accumulated on the PE, exact-width exp
            for kp in range(nfull, nkt, 2):
                sps = s_pool.tile([P, 1024], F32, tag="s", name="sps")
                for j, kt in enumerate((kp, kp + 1)):
                    off = (kt - nfull) * P
                    nc.tensor.matmul(
                        sps[:, j * 512 + off:(j + 1) * 512],
                        lhsT=kt_sb[:, h, kt * P:(kt + 1) * P],
                        rhs=qt_sb[:, h, q0 + off:q0 + 512],
                        start=True, stop=False, skip_group_check=True,
                    )
                    nc.tensor.matmul(
                        sps[:, j * 512 + off:j * 512 + off + P],
                        lhsT=i128, rhs=maskT,
                        start=False, stop=True, skip_group_check=True,
                    )
                for j, kt in enumerate((kp, kp + 1)):
                    off = (kt - nfull) * P
                    nc.scalar.activation(
                        out=ptg[:, kt, off:512],
                        in_=sps[:, j * 512 + off:(j + 1) * 512],
                        func=ACT_EXP, bias=0.0, scale=SCALE,
                    )
                yield

        # ---------------- stage 1: projections (V first, then Q/K) -------
        with tc.tile_pool(name="xw", bufs=1) as xw_pool, \
             tc.tile_pool(name="wqk", bufs=1) as wqk_pool, \
             tc.tile_pool(name="pjps", bufs=2, space="PSUM") as pj_psum:
            xt_sb = xw_pool.tile([P, NI, N_SEQ], BF16)
            wq_sb = wqk_pool.tile([P, NI, DC], BF16)
            wk_sb = wqk_pool.tile([P, NI, DC], BF16)

            # input DMAs in consumption order, ~0.26MB chunks (2 i-rows x
            # 512 cols) so the 16 DMA engines run in parallel
            def dma_w(dst, src):
                for i0 in range(0, NI, 2):
                    nc.sync.dma_start(dst[:, i0:i0 + 2, :], src[:, i0:i0 + 2, :])

            def dma_xt_win(w):
                c0, c1 = w * 512, (w + 1) * 512
                for i0 in range(0, NI, 2):
                    nc.sync.dma_start(xt_sb[:, i0:i0 + 2, c0:c1],
                                      xt_r[:, i0:i0 + 2, c0:c1])

            def emit_qk_pass(h, w_sb, dst, win):
                ps = pj_psum.tile([P, 512], F32, tag="qk")
                for i in range(NI):
                    nc.tensor.matmul(
                        ps,
                        lhsT=w_sb[:, i, h * P:(h + 1) * P],
                        rhs=xt_sb[:, i, win * 512:(win + 1) * 512],
                        start=(i == 0), stop=(i == NI - 1),
                    )
                nc.vector.tensor_copy(
                    out=dst[:, h, win * 512:(win + 1) * 512], in_=ps)

            # wv lives in an inner scope so its SBUF frees before the weave
            with tc.tile_pool(name="wvp", bufs=1) as wv_pool:
                wv_sb = wv_pool.tile([P, NI, DC], BF16)
                dma_w(wv_sb, wv_r)
                dma_xt_win(0)
                dma_xt_win(1)
                dma_w(wq_sb, wq_r)
                dma_w(wk_sb, wk_r)
                dma_xt_win(2)
                dma_xt_win(3)

                # dummy matmuls: keep the PE busy (and ramp the HAM clock
                # gate to full speed) while the first DMAs land
                warm_ps = pj_psum.tile([P, 512], F32, tag="v", name="warm_ps")
                for _ in range(40):
                    nc.tensor.matmul(warm_ps, lhsT=warmsrc[:, :P], rhs=warmsrc,
                                     start=True, stop=True)

                def emit_v(nt):
                    ps = pj_psum.tile([P, DC], F32, tag="v")
                    for i in range(NI):
                        nc.tensor.matmul(
                            ps,
                            lhsT=xt_sb[:, i, nt * P:(nt + 1) * P],
                            rhs=wv_sb[:, i, :],
                            start=(i == 0), stop=(i == NI - 1),
                        )
                    nc.vector.tensor_copy(out=v_sb[:, nt, :], in_=ps)

                # V-A, QK-A (windows 0,1), V-B
                for nt in range(0, 8):
                    emit_v(nt)
                for h in range(HPC):
                    for w_sb, dst in ((wq_sb, qt_sb), (wk_sb, kt_sb)):
                        for win in (0, 1):
                            emit_qk_pass(h, w_sb, dst, win)
                for nt in range(8, 16):
                    emit_v(nt)

            # QK-B (windows 2,3) with the first two g=3 score steps woven in
            weave = []  # generators of woven score units

            def pump_weave(n):
                for _ in range(n):
                    for gen in list(weave):
                        try:
                            next(gen)
                            break
                        except StopIteration:
                            weave.remove(gen)

            for h in range(HPC):
                for w_sb, dst in ((wq_sb, qt_sb), (wk_sb, kt_sb)):
                    for win in (2, 3):
                        emit_qk_pass(h, w_sb, dst, win)
                        pump_weave(2)
                if h == 0:
                    weave.append(emit_scores_units(3, 0, wv_ptg0, pj_psum))
            pump_weave(100)

        # ---------------- stage 2: attention + out-projection ------------
        with tc.tile_pool(name="att", bufs=4) as att_pool, \
             tc.tile_pool(name="small", bufs=2) as small_pool, \
             tc.tile_pool(name="osb", bufs=4) as out_pool, \
             tc.tile_pool(name="sps2", bufs=2, space="PSUM") as s_pool, \
             tc.tile_pool(name="colps", bufs=1, space="PSUM") as col_pool, \
             tc.tile_pool(name="ops", bufs=3, space="PSUM") as o_pool:
            wo_sb = att_pool.tile([P, HPC, D_IN], BF16, tag="wo", bufs=1)
            for hh in range(HPC):
                for j0 in (0, 1024):
                    nc.sync.dma_start(wo_sb[:, hh, j0:j0 + 1024],
                                      wo_r[:, hh, j0:j0 + 1024])

            ptgs = {(3, 0): wv_ptg0}

            def emit_colsum(g, h, ptg):
                """Column sums of P^T via a ones-row matmul; reciprocal is
                broadcast to all partitions for the fused ctx normalize."""
                nkt = 4 * (g + 1)
                colp = col_pool.tile([1, 512], F32, tag="col", name="colp")
                for kt in range(nkt):
                    off = max(kt - 4 * g, 0) * P
                    nc.tensor.matmul(
                        colp[:, off:512], lhsT=ones_sb, rhs=ptg[:, kt, off:512],
                        start=(kt == 0), stop=(kt == nkt - 1),
                        skip_group_check=True,
                    )
                recip_sb = small_pool.tile([1, 512], F32, tag="rsb",
                                           name="recip_sb")
                nc.vector.reciprocal_approx_fast(out=recip_sb, in_=colp)
                recip_bc = small_pool.tile([P, 512], F32, tag="rbc",
                                           name="recip_bc")
                nc.gpsimd.partition_broadcast(recip_bc, recip_sb)
                return recip_bc

            def emit_ctx(g, h, ptg, recip_bc):
                """ctx^T accumulated over k tiles, 1/colsum fused into the
                PSUM->SBUF copy."""
                nkt = 4 * (g + 1)
                cps = o_pool.tile([P, 512], F32, tag="o", name="cps")
                for kt in range(nkt):
                    off = max(kt - 4 * g, 0) * P
                    nc.tensor.matmul(
                        cps[:, off:512],
                        lhsT=v_sb[:, kt, h * P:(h + 1) * P],
                        rhs=ptg[:, kt, off:512],
                        start=(kt == 0), stop=(kt == nkt - 1),
                        skip_group_check=True,
                    )
                nc.vector.tensor_tensor(
                    out=ctxT_sb[:, h, g * 512:(g + 1) * 512],
                    in0=cps, in1=recip_bc, op=ALU.mult,
                )

            op_queue = []  # ready out-projection (nt, jc) tiles
            op_count = [0]

            def emit_op_tile(allow_act=False):
                nt, jc = op_queue.pop(0)
                ops = o_pool.tile([P, 512], F32, tag="o", name="ops")
                for hh in range(HPC):
                    nc.tensor.matmul(
                        ops,
                        lhsT=ctxT_sb[:, hh, nt * P:(nt + 1) * P],
                        rhs=wo_sb[:, hh, jc * 512:(jc + 1) * 512],
                        start=(hh == 0), stop=(hh == HPC - 1),
                    )
                osb = out_pool.tile([P, 512], F32, tag="osb", name="osb")
                op_count[0] += 1
                if allow_act and op_count[0] % 2 == 0:
                    nc.scalar.copy(out=osb, in_=ops)
                else:
                    nc.vector.tensor_copy(out=osb, in_=ops)
                nc.sync.dma_start(
                    out_d[nt * P:(nt + 1) * P, jc * 512:(jc + 1) * 512], osb)

            next_sc = 1  # (3,0) was woven into projection
            for idx, (g, h) in enumerate(steps):
                # emit scores up to LAG steps ahead, interleaved below
                gens = []
                while next_sc <= min(idx + LAG, len(steps) - 1):
                    g2, h2 = steps[next_sc]
                    ptg2 = att_pool.tile([P, NT, 512], BF16, tag="ptg",
                                         name=f"ptg_{g2}_{h2}")
                    ptgs[(g2, h2)] = ptg2
                    gens.append(emit_scores_units(g2, h2, ptg2, s_pool))
                    next_sc += 1

                def pump(n=1):
                    while n > 0 and gens:
                        try:
                            next(gens[0])
                            n -= 1
                        except StopIteration:
                            gens.pop(0)

                budget = 3 if idx < 8 else (5 if idx < 12 else 8)
                ptg = ptgs.pop((g, h))
                recip_bc = emit_colsum(g, h, ptg)
                pump(2)
                emit_ctx(g, h, ptg, recip_bc)
                # interleave remaining score units with out-proj filler
                for _ in range(10):
                    pump(1)
                    if budget > 0 and op_queue:
                        emit_op_tile(allow_act=(idx >= 6))
                        budget -= 1
                pump(100)
                while budget > 0 and op_queue:
                    emit_op_tile(allow_act=(idx >= 6))
                    budget -= 1
                if h == HPC - 1:
                    op_queue.extend(
                        (nt, jc) for nt in range(4 * g, 4 * g + 4)
                        for jc in range(NJ))
            while op_queue:
                emit_op_tile(allow_act=True)


def build_module():
    """Build and compile the per-core Bass module (SPMD: same program, 8 cores)."""
    nc = bacc.Bacc("TRN2", target_bir_lowering=False, debug=False,
                   num_devices=N_CORES)
    xt_d = nc.dram_tensor("xt", [D_IN, N_SEQ], BF16, kind="ExternalInput").ap()
    wq_d = nc.dram_tensor("wq", [D_IN, DC], BF16, kind="ExternalInput").ap()
    wk_d = nc.dram_tensor("wk", [D_IN, DC], BF16, kind="ExternalInput").ap()
    wv_d = nc.dram_tensor("wv", [D_IN, DC], BF16, kind="ExternalInput").ap()
    wo_d = nc.dram_tensor("wo", [DC, D_IN], BF16, kind="ExternalInput").ap()
    out_d = nc.dram_tensor("out", [N_SEQ, D_IN], F32, kind="ExternalOutput").ap()
    with tile.TileContext(nc) as tc:
        _build_body(tc, xt_d, wq_d, wk_d, wv_d, wo_d, out_d)
    nc.compile()
    return nc


def make_in_maps(x, W_qkv, W_out):
    """Host-side sharding: per-core input dict, bf16 cast + pre-transposed x."""
    bf = ml_dtypes.bfloat16
    in_maps = []
    for c in range(N_CORES):
        b, g = divmod(c, 4)
        cs = slice(DC * g, DC * (g + 1))
        in_maps.append({
            "xt": np.ascontiguousarray(x[b].T).astype(bf),
            "wq": np.ascontiguousarray(W_qkv[:, 0 * D_IN:1 * D_IN][:, cs]).astype(bf),
            "wk": np.ascontiguousarray(W_qkv[:, 1 * D_IN:2 * D_IN][:, cs]).astype(bf),
            "wv": np.ascontiguousarray(W_qkv[:, 2 * D_IN:3 * D_IN][:, cs]).astype(bf),
            "wo": np.ascontiguousarray(W_out[cs, :]).astype(bf),
        })
    return in_maps


_NC_CACHE = {}


def get_module():
    if "nc" not in _NC_CACHE:
        _NC_CACHE["nc"] = build_module()
    return _NC_CACHE["nc"]


def run(x, W_qkv, W_out, b_out, trace=False, **trace_kwargs):
    nc = get_module()
    in_maps = make_in_maps(x, W_qkv, W_out)
    res = run_bass_kernel_spmd(nc, in_maps, core_ids=list(range(N_CORES)),
                               trace=trace, **trace_kwargs)
    parts = np.stack([res.results[c]["out"] for c in range(N_CORES)])
    parts = parts.reshape(2, 4, N_SEQ, D_IN)
    out = parts.sum(axis=1, dtype=np.float64).astype(np.float32)
    out += b_out.astype(np.float32)
    return out, res


def kernel(x, W_qkv, W_out, b_out):
    out, _ = run(np.asarray(x), np.asarray(W_qkv), np.asarray(W_out),
                 np.asarray(b_out))
    return out


# revision 21
# speedup vs baseline: 1.0190x; 1.0190x over previous
"""Multi-head causal self-attention (B=2, N=2048, D=2048, H=16) on 8 NeuronCores.

Sharding: core c handles batch b = c//4 and heads 4*(c%4) .. 4*(c%4)+3
(data parallel over batch, tensor parallel over heads).  Each core:
  - projects V first (natural [seq, head_dim] layout), then Q^T / K^T in
    [head_dim, seq] layout, contracting over d_in chunks that sit on
    partitions (x is pre-transposed on the host),
  - runs causal attention per head entirely in transposed space
    (S^T = K_tile Q^T, exp on ScalarE writes P^T straight into SBUF,
    denominators via a ones-row matmul, 1/sum fused into the ctx copy),
  - computes the partial output projection ctx_slice @ W_out[rows_slice]
    into a [2048, 2048] fp32 partial.
The host sums the 4 partials per batch and adds the output bias.

v2 schedule: input DMAs are issued in consumption order in ~0.26 MB
chunks (16 DMA engines run in parallel; one huge DMA would serialize on
a single engine); the first two heads' g=3 score blocks + exp are woven
into the tail of the QK projection so the softmax pipeline is already
primed when projection ends; the causal mask rides the score PSUM
accumulation as an identity*maskT matmul (no cross-engine hop before
exp); exp is batched in [128,1024] pairs to amortize ACT access
latency; PSUM->SBUF out-tile copies run on DVE (ACT does only exp); and
out-projection tiles are interleaved as filler work between attention
steps so the PE never idles (idle costs ~3us of half-clock HAM ramp).

Matmul inputs are bf16 (fp32 accumulation in PSUM); measured end-to-end
relative error vs the fp32 reference is ~5.5e-3.
"""

import math

import numpy as np
import ml_dtypes

import concourse.bass as bass
import concourse.mybir as mybir
import concourse.tile as tile
from concourse import bacc
from concourse.bass_utils import run_bass_kernel_spmd

BF16 = mybir.dt.bfloat16
F32 = mybir.dt.float32
ALU = mybir.AluOpType
ACT_EXP = mybir.ActivationFunctionType.Exp

P = 128              # partitions
D_IN = 2048          # model dim
N_SEQ = 2048         # sequence length
HD = 128             # head dim
HPC = 4              # heads per core
DC = HPC * HD        # 512: d_out slice per core
N_CORES = 8
SCALE = 1.0 / math.sqrt(HD)
NEG_BIG = -1e10

NT = N_SEQ // P      # 16 seq tiles of 128
NI = D_IN // P       # 16 contraction chunks of 128
NG = NT // 4         # 4 groups of 4 q-tiles (q-512 groups)
NJ = D_IN // 512     # 4 output column chunks


def _build_body(tc, xt_d, wq_d, wk_d, wv_d, wo_d, out_d):
    nc = tc.nc
    from contextlib import ExitStack
    ctx = ExitStack()
    with ctx:
        # ---------------- constants ----------------
        const = ctx.enter_context(tc.tile_pool(name="const", bufs=1))
        ones_sb = const.tile([P, 1], BF16)
        nc.vector.memset(ones_sb, 1.0)
        warmsrc = const.tile([P, 512], BF16)
        nc.vector.memset(warmsrc, 0.0)
        # force the exp activation table load at t=0, off the critical path
        tscr = const.tile([P, 1], F32)
        nc.vector.memset(tscr, 0.0)
        nc.scalar.activation(out=tscr, in_=tscr, func=ACT_EXP, bias=0.0, scale=1.0)

        # ---------------- persistent activations ----------------
        persist = ctx.enter_context(tc.tile_pool(name="persist", bufs=1))
        qt_sb = persist.tile([P, HPC, N_SEQ], BF16)    # Q^T  [d, h, n]
        kt_sb = persist.tile([P, HPC, N_SEQ], BF16)    # K^T  [d, h, n]
        v_sb = persist.tile([P, NT, DC], BF16)         # V natural [n(128), nt, d]
        ctxT_sb = persist.tile([P, HPC, N_SEQ], BF16)  # ctx^T [d, h, n]
        # dedicated P^T tile for the score step woven into projection
        wv_ptg0 = persist.tile([P, NT, 512], BF16)

        xt_r = xt_d.rearrange("(io p) nn -> p io nn", p=P)
        wq_r = wq_d.rearrange("(io p) c -> p io c", p=P)
        wk_r = wk_d.rearrange("(io p) c -> p io c", p=P)
        wv_r = wv_d.rearrange("(io p) c -> p io c", p=P)
        wo_r = wo_d.rearrange("(h p) j -> p h j", p=P)

        # step list: groups in descending size so the pipeline has big work
        # first; (g, h) pairs in emission order
        steps = [(3 - i // HPC, i % HPC) for i in range(NG * HPC)]
        LAG = 2

        # ---------------- score/exp emitter (shared proj + main) ----------
        def emit_scores_units(g, h, ptg, s_pool):
            """Yield after each PE unit: full-kt pairs then diag pairs.
            Scores for group g head h: S^T = K_kt.T @ Q^T, mask accumulated
            on the PE for diagonal blocks, batched exp into ptg (bf16)."""
            nkt = 4 * (g + 1)
            q0 = 512 * g
            nfull = 4 * g
            # full-width kt tiles, two per [128,1024] psum tensor with one
            # batched exp (amortizes ACT access latency)
            for kp in range(0, nfull, 2):
                sps = s_pool.tile([P, 1024], F32, tag="s", name="sps")
                for j, kt in enumerate((kp, kp + 1)):
                    nc.tensor.matmul(
                        sps[:, j * 512:(j + 1) * 512],
                        lhsT=kt_sb[:, h, kt * P:(kt + 1) * P],
                        rhs=qt_sb[:, h, q0:q0 + 512],
                        start=True, stop=True,
                    )
                nc.scalar.activation(
                    out=ptg[:, kp:kp + 2, :], in_=sps,
                    func=ACT_EXP, bias=0.0, scale=SCALE,
                )
                yield
            # diagonal kt tiles (off = 128*(kt-4g)), two per psum tensor;
            # exp the full block (no overflow: |s*scale| <= ~9), then zero the
            # sub-diagonal triangle of P^T on the idle GpSimd engine
            for kp in range(nfull, nkt, 2):
                sps = s_pool.tile([P, 1024], F32, tag="s", name="sps")
                for j, kt in enumerate((kp, kp + 1)):
                    off = (kt - nfull) * P
                    nc.tensor.matmul(
                        sps[:, j * 512 + off:(j + 1) * 512],
                        lhsT=kt_sb[:, h, kt * P:(kt + 1) * P],
                        rhs=qt_sb[:, h, q0 + off:q0 + 512],
                        start=True, stop=True,
                    )
                for j, kt in enumerate((kp, kp + 1)):
                    off = (kt - nfull) * P
                    nc.scalar.activation(
                        out=ptg[:, kt, off:512],
                        in_=sps[:, j * 512 + off:(j + 1) * 512],
                        func=ACT_EXP, bias=0.0, scale=SCALE,
                    )
                for j, kt in enumerate((kp, kp + 1)):
                    off = (kt - nfull) * P
                    nc.gpsimd.affine_select(
                        out=ptg[:, kt, off:off + P], in_=ptg[:, kt, off:off + P],
                        compare_op=ALU.is_ge, fill=0.0,
                        base=0, pattern=[[1, P]], channel_multiplier=-1,
                    )
                yield

        # ---------------- stage 1: projections (V first, then Q/K) -------
        with tc.tile_pool(name="xw", bufs=1) as xw_pool, \
             tc.tile_pool(name="wqk", bufs=1) as wqk_pool, \
             tc.tile_pool(name="pjps", bufs=2, space="PSUM") as pj_psum:
            xt_sb = xw_pool.tile([P, NI, N_SEQ], BF16)
            wq_sb = wqk_pool.tile([P, NI, DC], BF16)
            wk_sb = wqk_pool.tile([P, NI, DC], BF16)

            # input DMAs in consumption order, ~0.26MB chunks (2 i-rows x
            # 512 cols) so the 16 DMA engines run in parallel
            def dma_w(dst, src):
                for i0 in range(0, NI, 2):
                    nc.sync.dma_start(dst[:, i0:i0 + 2, :], src[:, i0:i0 + 2, :])

            def dma_xt_win(w):
                c0, c1 = w * 512, (w + 1) * 512
                for i0 in range(0, NI, 2):
                    nc.sync.dma_start(xt_sb[:, i0:i0 + 2, c0:c1],
                                      xt_r[:, i0:i0 + 2, c0:c1])

            def emit_qk_pass(h, w_sb, dst, win):
                ps = pj_psum.tile([P, 512], F32, tag="qk")
                for i in range(NI):
                    nc.tensor.matmul(
                        ps,
                        lhsT=w_sb[:, i, h * P:(h + 1) * P],
                        rhs=xt_sb[:, i, win * 512:(win + 1) * 512],
                        start=(i == 0), stop=(i == NI - 1),
                    )
                nc.vector.tensor_copy(
                    out=dst[:, h, win * 512:(win + 1) * 512], in_=ps)

            # wv lives in an inner scope so its SBUF frees before the weave
            with tc.tile_pool(name="wvp", bufs=1) as wv_pool:
                wv_sb = wv_pool.tile([P, NI, DC], BF16)
                dma_w(wv_sb, wv_r)
                dma_xt_win(0)
                dma_xt_win(1)
                dma_w(wq_sb, wq_r)
                dma_w(wk_sb, wk_r)
                dma_xt_win(2)
                dma_xt_win(3)

                # dummy matmuls: keep the PE busy (and ramp the HAM clock
                # gate to full speed) while the first DMAs land
                warm_ps = pj_psum.tile([P, 512], F32, tag="v", name="warm_ps")
                for _ in range(40):
                    nc.tensor.matmul(warm_ps, lhsT=warmsrc[:, :P], rhs=warmsrc,
                                     start=True, stop=True)

                def emit_v(nt):
                    ps = pj_psum.tile([P, DC], F32, tag="v")
                    for i in range(NI):
                        nc.tensor.matmul(
                            ps,
                            lhsT=xt_sb[:, i, nt * P:(nt + 1) * P],
                            rhs=wv_sb[:, i, :],
                            start=(i == 0), stop=(i == NI - 1),
                        )
                    nc.vector.tensor_copy(out=v_sb[:, nt, :], in_=ps)

                # V-A, QK-A (windows 0,1), V-B
                for nt in range(0, 8):
                    emit_v(nt)
                for h in range(HPC):
                    for w_sb, dst in ((wq_sb, qt_sb), (wk_sb, kt_sb)):
                        for win in (0, 1):
                            emit_qk_pass(h, w_sb, dst, win)
                for nt in range(8, 16):
                    emit_v(nt)

            # QK-B (windows 2,3) with the first two g=3 score steps woven in
            weave = []  # generators of woven score units

            def pump_weave(n):
                for _ in range(n):
                    for gen in list(weave):
                        try:
                            next(gen)
                            break
                        except StopIteration:
                            weave.remove(gen)

            for h in range(HPC):
                for w_sb, dst in ((wq_sb, qt_sb), (wk_sb, kt_sb)):
                    for win in (2, 3):
                        emit_qk_pass(h, w_sb, dst, win)
                        pump_weave(2)
                if h == 0:
                    weave.append(emit_scores_units(3, 0, wv_ptg0, pj_psum))
            pump_weave(100)

        # ---------------- stage 2: attention + out-projection ------------
        with tc.tile_pool(name="att", bufs=4) as att_pool, \
             tc.tile_pool(name="small", bufs=2) as small_pool, \
             tc.tile_pool(name="osb", bufs=4) as out_pool, \
             tc.tile_pool(name="sps2", bufs=2, space="PSUM") as s_pool, \
             tc.tile_pool(name="colps", bufs=1, space="PSUM") as col_pool, \
             tc.tile_pool(name="ops", bufs=3, space="PSUM") as o_pool:
            wo_sb = att_pool.tile([P, HPC, D_IN], BF16, tag="wo", bufs=1)
            for hh in range(HPC):
                for j0 in (0, 1024):
                    nc.sync.dma_start(wo_sb[:, hh, j0:j0 + 1024],
                                      wo_r[:, hh, j0:j0 + 1024])

            ptgs = {(3, 0): wv_ptg0}

            def emit_colsum(g, h, ptg):
                """Column sums of P^T via a ones-row matmul; reciprocal is
                broadcast to all partitions for the fused ctx normalize."""
                nkt = 4 * (g + 1)
                colp = col_pool.tile([1, 512], F32, tag="col", name="colp")
                for kt in range(nkt):
                    off = max(kt - 4 * g, 0) * P
                    nc.tensor.matmul(
                        colp[:, off:512], lhsT=ones_sb, rhs=ptg[:, kt, off:512],
                        start=(kt == 0), stop=(kt == nkt - 1),
                        skip_group_check=True,
                    )
                recip_sb = small_pool.tile([1, 512], F32, tag="rsb",
                                           name="recip_sb")
                nc.vector.reciprocal_approx_fast(out=recip_sb, in_=colp)
                recip_bc = small_pool.tile([P, 512], F32, tag="rbc",
                                           name="recip_bc")
                nc.gpsimd.partition_broadcast(recip_bc, recip_sb)
                return recip_bc

            def emit_ctx(g, h, ptg, recip_bc):
                """ctx^T accumulated over k tiles, 1/colsum fused into the
                PSUM->SBUF copy."""
                nkt = 4 * (g + 1)
                cps = o_pool.tile([P, 512], F32, tag="o", name="cps")
                for kt in range(nkt):
                    off = max(kt - 4 * g, 0) * P
                    nc.tensor.matmul(
                        cps[:, off:512],
                        lhsT=v_sb[:, kt, h * P:(h + 1) * P],
                        rhs=ptg[:, kt, off:512],
                        start=(kt == 0), stop=(kt == nkt - 1),
                        skip_group_check=True,
                    )
                nc.vector.tensor_tensor(
                    out=ctxT_sb[:, h, g * 512:(g + 1) * 512],
                    in0=cps, in1=recip_bc, op=ALU.mult,
                )

            op_queue = []  # ready out-projection (nt, jc) tiles
            op_count = [0]

            def emit_op_tile(allow_act=False):
                nt, jc = op_queue.pop(0)
                ops = o_pool.tile([P, 512], F32, tag="o", name="ops")
                for hh in range(HPC):
                    nc.tensor.matmul(
                        ops,
                        lhsT=ctxT_sb[:, hh, nt * P:(nt + 1) * P],
                        rhs=wo_sb[:, hh, jc * 512:(jc + 1) * 512],
                        start=(hh == 0), stop=(hh == HPC - 1),
                    )
                osb = out_pool.tile([P, 512], F32, tag="osb", name="osb")
                op_count[0] += 1
                if allow_act and op_count[0] % 2 == 0:
                    nc.scalar.copy(out=osb, in_=ops)
                else:
                    nc.vector.tensor_copy(out=osb, in_=ops)
                nc.sync.dma_start(
                    out_d[nt * P:(nt + 1) * P, jc * 512:(jc + 1) * 512], osb)

            next_sc = 1  # (3,0) was woven into projection
            for idx, (g, h) in enumerate(steps):
                # emit scores up to LAG steps ahead, interleaved below
                gens = []
                while next_sc <= min(idx + LAG, len(steps) - 1):
                    g2, h2 = steps[next_sc]
                    ptg2 = att_pool.tile([P, NT, 512], BF16, tag="ptg",
                                         name=f"ptg_{g2}_{h2}")
                    ptgs[(g2, h2)] = ptg2
                    gens.append(emit_scores_units(g2, h2, ptg2, s_pool))
                    next_sc += 1

                def pump(n=1):
                    while n > 0 and gens:
                        try:
                            next(gens[0])
                            n -= 1
                        except StopIteration:
                            gens.pop(0)

                budget = 4 if idx < 8 else 7
                ptg = ptgs.pop((g, h))
                recip_bc = emit_colsum(g, h, ptg)
                pump(2)
                emit_ctx(g, h, ptg, recip_bc)
                # interleave remaining score units with out-proj filler
                for _ in range(10):
                    pump(1)
                    if budget > 0 and op_queue:
                        emit_op_tile(allow_act=(idx >= 6))
                        budget -= 1
                pump(100)
                while budget > 0 and op_queue:
                    emit_op_tile(allow_act=(idx >= 6))
                    budget -= 1
                if h == HPC - 1:
                    op_queue.extend(
                        (nt, jc) for nt in range(4 * g, 4 * g + 4)
                        for jc in range(NJ))
            while op_queue:
                emit_op_tile(allow_act=True)


def build_module():
    """Build and compile the per-core Bass module (SPMD: same program, 8 cores)."""
    nc = bacc.Bacc("TRN2", target_bir_lowering=False, debug=False,
                   num_devices=N_CORES)
    xt_d = nc.dram_tensor("xt", [D_IN, N_SEQ], BF16, kind="ExternalInput").ap()
    wq_d = nc.dram_tensor("wq", [D_IN, DC], BF16, kind="ExternalInput").ap()
    wk_d = nc.dram_tensor("wk", [D_IN, DC], BF16, kind="ExternalInput").ap()
    wv_d = nc.dram_tensor("wv", [D_IN, DC], BF16, kind="ExternalInput").ap()
    wo_d = nc.dram_tensor("wo", [DC, D_IN], BF16, kind="ExternalInput").ap()
    out_d = nc.dram_tensor("out", [N_SEQ, D_IN], F32, kind="ExternalOutput").ap()
    with tile.TileContext(nc) as tc:
        _build_body(tc, xt_d, wq_d, wk_d, wv_d, wo_d, out_d)
    nc.compile()
    return nc


def make_in_maps(x, W_qkv, W_out):
    """Host-side sharding: per-core input dict, bf16 cast + pre-transposed x."""
    bf = ml_dtypes.bfloat16
    in_maps = []
    for c in range(N_CORES):
        b, g = divmod(c, 4)
        cs = slice(DC * g, DC * (g + 1))
        in_maps.append({
            "xt": np.ascontiguousarray(x[b].T).astype(bf),
            "wq": np.ascontiguousarray(W_qkv[:, 0 * D_IN:1 * D_IN][:, cs]).astype(bf),
            "wk": np.ascontiguousarray(W_qkv[:, 1 * D_IN:2 * D_IN][:, cs]).astype(bf),
            "wv": np.ascontiguousarray(W_qkv[:, 2 * D_IN:3 * D_IN][:, cs]).astype(bf),
            "wo": np.ascontiguousarray(W_out[cs, :]).astype(bf),
        })
    return in_maps


_NC_CACHE = {}


def get_module():
    if "nc" not in _NC_CACHE:
        _NC_CACHE["nc"] = build_module()
    return _NC_CACHE["nc"]


def run(x, W_qkv, W_out, b_out, trace=False, **trace_kwargs):
    nc = get_module()
    in_maps = make_in_maps(x, W_qkv, W_out)
    res = run_bass_kernel_spmd(nc, in_maps, core_ids=list(range(N_CORES)),
                               trace=trace, **trace_kwargs)
    parts = np.stack([res.results[c]["out"] for c in range(N_CORES)])
    parts = parts.reshape(2, 4, N_SEQ, D_IN)
    out = parts.sum(axis=1, dtype=np.float64).astype(np.float32)
    out += b_out.astype(np.float32)
    return out, res


def kernel(x, W_qkv, W_out, b_out):
    out, _ = run(np.asarray(x), np.asarray(W_qkv), np.asarray(W_out),
                 np.asarray(b_out))
    return out
